# revision 1
# baseline (speedup 1.0000x reference)
"""Trainium2 Bass kernel for nn_CrossAttention (B=4, L=2048, Da=Db=H=256).

Math (per batch b):
  mu = input_a @ Wa + ba ; mv = input_b @ Wb + bb
  S[l, m] = mu[l] . mv[m]
  output_a[l, d] = sum_m exp(S[l,m]) / (sum_l' exp(S[l',m])) * input_b[m, d]
  output_b[m, d] = sum_l exp(S[l,m]) / (sum_m' exp(S[l,m'])) * input_a[l, d]
  out = concat([output_a, padding_values, output_b], axis=-1)

Both outputs are the same generic function g with operands swapped:
  g(U, V, Wu, bu, Wv, bv)[j, d] = sum_i (E[i,j] / R[i]) * U[i, d]
    where E = exp((U@Wu+bu) @ (V@Wv+bv)^T), R[i] = sum_j E[i, j]
  output_b[b] = g(input_a[b], input_b[b], Wa, ba, Wb, bb)
  output_a[b] = g(input_b[b], input_a[b], Wb, bb, Wa, ba)

Sharding: 8 cores = 4 batches x 2 roles; every core runs the SAME program
with different input bindings (pure SPMD, no collectives). padding_values
passes through on the host.

Schedule (single core): eagerly pipelined --
  - one DMA queue (SP), ordered [weights, U blk0, V chunks, U chunks]:
    the transfer pipe is a single serialized resource, and V gates
    every S column while the weights let projections chase V arrivals.
  - pre-phase: warm-up (PE p-state ramp) -> V^T/U^T transposes and
    mv^T/mu^T projections chase the chunk arrivals through a deep
    4-buffer staging pool; block 0's S halves run here from a
    dedicated 2-bank pool as each mv^T half lands.
  - phase 1: S row blocks at the PE-bound period; output matmuls for
    j-tiles 0..7 stream with a TWO-block lag so the DVE row-sum
    latency stays off the critical loop; U chunks 2..4 stage through
    a small side pool during the early, DMA-gated blocks (j4-7 join
    at block K_CATCHUP with spread catch-up matmuls).
  - phase 2: j-tiles 8..15 from the cached E on the retired S/acc
    banks; the last four as single-j accumulators (start=True clears)
    with per-j drains overlapping the next j's matmuls.
  - dependency tracking is tile-granular, so U/V/E/Ut are split into
    per-chunk (U, V) and block-interleaved (E, Ut) tiles: one big tile
    would serialize pipeline stages through false WAR hazards.
  - row sum R: half 0 rides the exp on ACT (accum_out); half 1 is a
    DVE tensor_reduce over bf16 E -- balances ACT vs DVE per block.
  - transposes run as float32r (1.5 cycles/row vs 2.0 for fp32); all
    f32r matmul operands are produced rounded (DVE copies / DMA into
    f32r tiles) to satisfy the BIR verifier.

Numerics: scores stay in [-65, 65] for this problem's distribution
(checked empirically, inputs ~N(0,1) with 0.05-scaled weights), so exp()
without max-subtraction is safe in fp32. Matmuls run as float32r
(full-rate at N>=256); E and the row-normalized U are stored bf16 for the
second matmul pass. Validated end-to-end scale-relative error ~2e-3.
"""

import sys
from contextlib import ExitStack

import numpy as np

for _p in ("/opt/trn_rl_repo", "/opt/pypackages"):
    if _p not in sys.path:
        sys.path.append(_p)

import concourse.bass as bass  # noqa: E402
import concourse.tile as tile  # noqa: E402
from concourse import bacc, mybir  # noqa: E402
from concourse.bass_utils import run_bass_kernel_spmd  # noqa: E402
from concourse.masks import make_identity  # noqa: E402

B, L, D, H = 4, 2048, 256, 256
NBLK = L // 128  # 16 row blocks
F32 = mybir.dt.float32
F32R = mybir.dt.float32r
BF16 = mybir.dt.bfloat16
FT = mybir.ActivationFunctionType
AX = mybir.AxisListType
ALU = mybir.AluOpType

WARM_N = 7       # warm-up matmuls (PE p-state ramp covers ~3us)
K_CATCHUP = 8    # block at which acc pair j4-7 joins (staging pool closed)

_BUILT = {}


def _build():
    if "nc" in _BUILT:
        return _BUILT

    nc = bacc.Bacc("TRN2", target_bir_lowering=False, debug=False)

    U_d = nc.dram_tensor("U", [L, D], F32, kind="ExternalInput").ap()
    V_d = nc.dram_tensor("V", [L, D], F32, kind="ExternalInput").ap()
    Wu_d = nc.dram_tensor("Wu", [D, H], F32, kind="ExternalInput").ap()
    bu_d = nc.dram_tensor("bu", [H], F32, kind="ExternalInput").ap()
    Wv_d = nc.dram_tensor("Wv", [D, H], F32, kind="ExternalInput").ap()
    bv_d = nc.dram_tensor("bv", [H], F32, kind="ExternalInput").ap()
    out_d = nc.dram_tensor("out", [L, D], F32, kind="ExternalOutput").ap()

    with ExitStack() as ctx:
        tc = ctx.enter_context(tile.TileContext(nc))

        sb = ctx.enter_context(tc.tile_pool(name="sb", bufs=1))
        io = ctx.enter_context(tc.tile_pool(name="io", bufs=2))

        # ---- persistent SBUF tensors ----
        # U/V split per DMA chunk, E/Ut interleaved by block (i % 4):
        # tile-granular dep tracking would otherwise serialize pipeline
        # stages through false WAR hazards on one big tile.
        Uc0 = sb.tile([128, 1, D], F32R, name="Uc0", tag="Uc0")
        Uc1 = sb.tile([128, 3, D], F32R, name="Uc1", tag="Uc1")
        Uc = [Uc0, Uc1] + [
            sb.tile([128, 4, D], F32R, name=f"Uc{k}", tag=f"Uc{k}")
            for k in range(2, 5)
        ]
        Vc0a = sb.tile([128, 2, D], F32R, name="Vc0a", tag="Vc0a")
        Vc0b = sb.tile([128, 2, D], F32R, name="Vc0b", tag="Vc0b")
        Vc = [None] + [sb.tile([128, 4, D], F32R, name=f"Vc{k}", tag=f"Vc{k}")
                       for k in range(1, 4)]
        Eb = [sb.tile([128, 4, L], BF16, name=f"Eb{k}", tag=f"Eb{k}")
              for k in range(4)]
        Utb = [sb.tile([128, 4, D], BF16, name=f"Utb{k}", tag=f"Utb{k}")
               for k in range(4)]
        UT_sb = sb.tile([128, 2, L], F32R, tag="UT")     # U^T, d on partitions
        VT_sb = sb.tile([128, 2, L], F32R, tag="VT")
        muT_sb = sb.tile([128, 2, L], F32R, tag="muT")   # mu^T, h on partitions
        mvT_sb = sb.tile([128, 2, L], F32R, tag="mvT")
        R_sb = sb.tile([128, NBLK], F32, tag="R")
        Ri_sb = sb.tile([128, NBLK], F32, tag="Ri")
        Rh_sb = sb.tile([128, NBLK, 2], F32, tag="Rh")
        Wu_sb = sb.tile([128, 2, H], F32, tag="Wu")
        Wv_sb = sb.tile([128, 2, H], F32, tag="Wv")
        bu_sb = sb.tile([128, 2], F32, tag="bu")
        bv_sb = sb.tile([128, 2], F32, tag="bv")
        ident = sb.tile([128, 128], F32, tag="ident")
        identr = sb.tile([128, 128], F32R, tag="identr")
        zeros_sb = sb.tile([128, 512], F32, tag="zeros")
        zerr = sb.tile([128, 512], F32R, tag="zerr")
        Wur_sb = sb.tile([128, 2, H], F32R, tag="Wur")
        Wvr_sb = sb.tile([128, 2, H], F32R, tag="Wvr")

        def u_blk(i):
            """SBUF slice [128, 256] holding U rows-block i."""
            if i == 0:
                return Uc[0][:, 0, :]
            if i < 4:
                return Uc[1][:, i - 1, :]
            return Uc[i // 4 + 1][:, i % 4, :]

        def v_blk(i):
            if i < 2:
                return Vc0a[:, i, :]
            if i < 4:
                return Vc0b[:, i - 2, :]
            return Vc[i // 4][:, i % 4, :]

        def e_blk(i):
            """SBUF slice [128, 2048] for E of rows-block i."""
            return Eb[i % 4][:, i // 4, :]

        def ut_blk(i):
            return Utb[i % 4][:, i // 4, :]

        U_view = U_d.rearrange("(t p) d -> p t d", p=128)
        V_view = V_d.rearrange("(t p) d -> p t d", p=128)
        out_view = out_d.rearrange("(t p) d -> p t d", p=128)

        # ---- input DMAs, single SP queue; order = arrival priority.
        # The transfer pipe is a single serialized resource, so V (which
        # gates every S column) goes first; weights follow (projections
        # start only once V^T exists anyway). ----
        sy = nc.sync
        sy.dma_start(Wv_sb[:], Wv_d.rearrange("(s p) h -> p s h", p=128))
        sy.dma_start(bv_sb[:], bv_d.rearrange("(s p) -> p s", p=128))
        sy.dma_start(Uc[0][:], U_view[:, 0:1, :].bitcast(F32R))
        sy.dma_start(Wu_sb[:], Wu_d.rearrange("(s p) h -> p s h", p=128))
        sy.dma_start(bu_sb[:], bu_d.rearrange("(s p) -> p s", p=128))
        sy.dma_start(Vc0a[:], V_view[:, 0:2, :].bitcast(F32R))
        sy.dma_start(Vc0b[:], V_view[:, 2:4, :].bitcast(F32R))
        for c in range(1, 4):
            sy.dma_start(Vc[c][:], V_view[:, 4 * c:4 * c + 4, :].bitcast(F32R))
        sy.dma_start(Uc[1][:], U_view[:, 1:4, :].bitcast(F32R))
        for c in range(1, 4):
            sy.dma_start(Uc[c + 1][:], U_view[:, 4 * c:4 * c + 4, :].bitcast(F32R))

        nc.vector.memset(zeros_sb[:], 0.0)
        make_identity(nc, ident[:])
        # f32r operands must be produced rounded (BIR verifier); DVE
        # copies are the canonical legal f32r producers
        nc.vector.tensor_copy(zerr[:], zeros_sb[:])
        nc.vector.tensor_copy(identr[:], ident[:])
        nc.vector.tensor_copy(Wur_sb[:], Wu_sb[:])
        nc.vector.tensor_copy(Wvr_sb[:], Wv_sb[:])

        idr = identr[:]
        copy_flip = [0]

        def alt_copy(dst, src):
            # alternate PSUM->SBUF moves between DVE and ACT
            if copy_flip[0] % 2 == 0:
                nc.vector.tensor_copy(dst, src)
            else:
                nc.scalar.copy(dst, src)
            copy_flip[0] += 1

        def transp_blocks(pt_pool, xT_sb, blks, blk_ap):
            """Transpose rows-blocks into xT. One staging tile per d-half;
            the two halves' copies run concurrently on DVE and ACT."""
            n = len(blks)
            for dh in range(2):
                st = pt_pool.tile([128, 512], F32, tag="pt")
                for k, blk in enumerate(blks):
                    nc.tensor.transpose(
                        st[:, k * 128:(k + 1) * 128].bitcast(F32R),
                        blk_ap(blk)[:, dh * 128:(dh + 1) * 128],
                        idr,
                    )
                dst = xT_sb[:, dh, blks[0] * 128:(blks[0] + n) * 128]
                if dh == 0:
                    nc.vector.tensor_copy(dst, st[:, 0:n * 128])
                else:
                    nc.scalar.copy(dst, st[:, 0:n * 128])

        def proj_unit(pt_pool, W_sb, b_sb, xT_sb, mT, hh, off, size):
            """mT[:, hh, off:off+size] = (W^T x^T + b) for one h-half/span."""
            pt = pt_pool.tile([128, 512], F32, tag="pt")
            for s in range(2):
                nc.tensor.matmul(
                    pt[:, 0:size],
                    W_sb[:, s, hh * 128:(hh + 1) * 128],
                    xT_sb[:, s, off:off + size],
                    start=(s == 0),
                    stop=(s == 1),
                )
            dst = mT[:, hh, off:off + size]
            if hh == 0:
                nc.vector.tensor_scalar_add(dst, pt[:, 0:size], b_sb[:, hh:hh + 1])
            else:
                nc.scalar.activation(
                    dst, pt[:, 0:size], FT.Identity, bias=b_sb[:, hh:hh + 1]
                )

        def s_half(ps_pool, i, half):
            ps = ps_pool.tile([128, 1024], F32, tag="ps")
            for chk in range(2):
                for hh in range(2):
                    nc.tensor.matmul(
                        ps[:, chk * 512:(chk + 1) * 512],
                        muT_sb[:, hh, i * 128:(i + 1) * 128],
                        mvT_sb[:, hh, half * 1024 + chk * 512:
                               half * 1024 + (chk + 1) * 512],
                        start=(hh == 0),
                        stop=(hh == 1),
                    )
            if half == 0:
                # row-sum of this half rides the exp (ACT accumulator)
                nc.scalar.activation(
                    e_blk(i)[:, 0:1024], ps[:], FT.Exp,
                    accum_out=Rh_sb[:, i, 0:1],
                )
            else:
                nc.scalar.activation(e_blk(i)[:, 1024:2048], ps[:], FT.Exp)

        def finish_block(i):
            # half-1 row sums on DVE from bf16 E; half-0 came via ACT accum
            nc.vector.tensor_reduce(
                Rh_sb[:, i, 1:2], e_blk(i)[:, 1024:2048], AX.X, ALU.add
            )
            nc.vector.tensor_add(
                R_sb[:, i:i + 1], Rh_sb[:, i, 0:1], Rh_sb[:, i, 1:2]
            )
            nc.vector.reciprocal(Ri_sb[:, i:i + 1], R_sb[:, i:i + 1])
            nc.vector.tensor_scalar_mul(
                ut_blk(i), u_blk(i).bitcast(F32), Ri_sb[:, i:i + 1]
            )

        def clear_acc(acc_t):
            nc.tensor.matmul(
                acc_t.rearrange("p a d -> p (a d)"),
                zerr[:, 0:128],
                zerr[:],
                start=True,
                stop=False,
                skip_group_check=True,
            )

        def out_mm(i, j, acc_ap):
            nc.tensor.matmul(
                acc_ap,
                e_blk(i)[:, j * 128:(j + 1) * 128],
                ut_blk(i),
                start=False,
                stop=(i == NBLK - 1),
                skip_group_check=True,
            )

        def out_mms(i, js, accpairs, j0):
            for j in js:
                out_mm(i, j, accpairs[(j - j0) // 2][:, (j - j0) % 2, :])

        def drain_pair(acc_t, p):
            ot = io.tile([128, 2, D], F32, name=f"ot{p}", tag="ot")
            alt_copy(ot[:], acc_t[:])
            nc.sync.dma_start(out_view[:, 2 * p:2 * p + 2, :], ot[:])

        acc1 = ctx.enter_context(
            tc.tile_pool(name="acc1", bufs=2, space="PSUM")
        )
        accs_a = [acc1.tile([128, 2, D], F32, name=f"acca{k}", tag="acc")
                  for k in range(2)]

        ps_scope = ExitStack()
        acc2_scope = ExitStack()
        ps0_scope = ExitStack()
        ps0_pool = ps0_scope.enter_context(
            tc.tile_pool(name="ps0", bufs=1, space="PSUM")
        )
        # pre-phase: deep staging pool (4 banks; ps/acc banks are not
        # needed yet) so V chunks transpose/project truly pipelined
        with tc.tile_pool(name="ptv", bufs=4, space="PSUM") as ptv:
            # PE warm-up: p-state ramp while the first DMAs stream
            warm = ptv.tile([128, 512], F32, tag="pt")
            for w in range(WARM_N):
                nc.tensor.matmul(
                    warm[:],
                    zerr[:, 0:128],
                    zerr[:],
                    start=True,
                    stop=(w == WARM_N - 1),
                    skip_group_check=True,
                )
            for a in accs_a:
                clear_acc(a)

            # U block 0 + V chunks: each chunk's mv^T projection follows
            # its transposes immediately (weights are already resident),
            # so the projection pipeline chases the V arrivals
            transp_blocks(ptv, UT_sb, [0], u_blk)
            for c in range(4):
                transp_blocks(ptv, VT_sb, [4 * c, 4 * c + 1], v_blk)
                transp_blocks(ptv, VT_sb, [4 * c + 2, 4 * c + 3], v_blk)
                if c == 0:
                    for hh in range(2):
                        proj_unit(ptv, Wur_sb, bu_sb, UT_sb, muT_sb,
                                  hh, 0, 128)
                for hh in range(2):
                    proj_unit(ptv, Wvr_sb, bv_sb, VT_sb, mvT_sb,
                              hh, 512 * c, 512)
                # block 0's S halves run inside the pre-phase from a
                # dedicated 2-bank pool, right as each mv^T half lands
                if c == 2:
                    s_half(ps0_pool, 0, 0)
            s_half(ps0_pool, 0, 1)
            # U rows 1:4 -> mu^T cols 128:512
            transp_blocks(ptv, UT_sb, [1, 2, 3], u_blk)
            for hh in range(2):
                proj_unit(ptv, Wur_sb, bu_sb, UT_sb, muT_sb, hh, 128, 384)
            finish_block(0)

        ps0_scope.close()
        ps_pool = ps_scope.enter_context(
            tc.tile_pool(name="ps", bufs=2, space="PSUM")
        )

        with tc.tile_pool(name="ptu", bufs=2, space="PSUM") as ptu:

            # blocks 1..K_CATCHUP-1 with j0-3 streaming + U staging spread
            # (outs lag three blocks so the DVE row-sum latency stays off
            # the critical loop)
            for i in range(1, K_CATCHUP):
                s_half(ps_pool, i, 0)
                s_half(ps_pool, i, 1)
                finish_block(i)
                if i >= 3:
                    out_mms(i - 3, [0, 1, 2, 3], accs_a, 0)
                # stage U chunks 2..4: transposes one block, projs the next
                if i in (1, 3, 5):
                    uc = (i + 1) // 2
                    transp_blocks(ptu, UT_sb,
                                  [4 * uc, 4 * uc + 1], u_blk)
                    transp_blocks(ptu, UT_sb,
                                  [4 * uc + 2, 4 * uc + 3], u_blk)
                elif i in (2, 4, 6):
                    uc = i // 2
                    for hh in range(2):
                        proj_unit(ptu, Wur_sb, bu_sb, UT_sb, muT_sb,
                                  hh, 512 * uc, 512)

        # staging banks hand over to acc pair j4-7
        acc2 = acc2_scope.enter_context(
            tc.tile_pool(name="acc2", bufs=2, space="PSUM")
        )
        accs_b = [acc2.tile([128, 2, D], F32, name=f"accb{k}", tag="acc")
                  for k in range(2)]
        for a in accs_b:
            clear_acc(a)

        for i in range(K_CATCHUP, NBLK):
            s_half(ps_pool, i, 0)
            s_half(ps_pool, i, 1)
            finish_block(i)
            out_mms(i - 3, [0, 1, 2, 3], accs_a, 0)
            out_mms(i - 3, [4, 5, 6, 7], accs_b, 4)
            # catch-up for j4-7 (blocks 0..K-3), spread over three blocks
            if i - K_CATCHUP < 3:
                k0 = 2 * (i - K_CATCHUP)
                for ci in range(k0, min(k0 + 2, K_CATCHUP - 3)):
                    out_mms(ci, [4, 5, 6, 7], accs_b, 4)

        for i in (NBLK - 3, NBLK - 2, NBLK - 1):
            out_mms(i, [0, 1, 2, 3], accs_a, 0)
            out_mms(i, [4, 5, 6, 7], accs_b, 4)

        # drain streamed accumulators as 4-row units (one descriptor-gen
        # each; ACT and DVE split the copies)
        ota = io.tile([128, 4, D], F32, name="ota", tag="ot4")
        nc.scalar.copy(ota[:, 0:2, :], accs_a[0][:])
        nc.scalar.copy(ota[:, 2:4, :], accs_a[1][:])
        nc.sync.dma_start(out_view[:, 0:4, :], ota[:])
        otb = io.tile([128, 4, D], F32, name="otb", tag="ot4")
        nc.vector.tensor_copy(otb[:, 0:2, :], accs_b[0][:])
        nc.vector.tensor_copy(otb[:, 2:4, :], accs_b[1][:])
        nc.sync.dma_start(out_view[:, 4:8, :], otb[:])

        # ---- phase 2: ps + acc2 banks retire to j8-15 accumulators ----
        acc2_scope.close()
        ps_scope.close()
        with tc.tile_pool(name="pp2", bufs=4, space="PSUM") as pp2:
            for jp in range(4, 6):
                acc = pp2.tile([128, 2, D], F32, name=f"accc{jp}", tag="acc")
                clear_acc(acc)
                for i in range(NBLK):
                    for j in (2 * jp, 2 * jp + 1):
                        out_mm(i, j, acc[:, j % 2, :])
                drain_pair(acc, jp)

            # j-tiles 12..15: single-j accumulators; start=True on the
            # first matmul replaces a zeroing pass (rest of bank unused),
            # and each drain overlaps the next j's matmuls.
            for jj in range(4):
                j = 12 + jj
                acc = pp2.tile([128, 2, D], F32, name=f"accd{jj}", tag="acc")
                nc.tensor.matmul(
                    acc[:, 0, :],
                    e_blk(0)[:, j * 128:(j + 1) * 128],
                    ut_blk(0),
                    start=True,
                    stop=False,
                    skip_group_check=True,
                )
                for i in range(1, NBLK):
                    out_mm(i, j, acc[:, 0, :])
                ot = io.tile([128, D], F32, name=f"otj{jj}", tag="oh")
                if jj % 2 == 0:
                    nc.scalar.copy(ot[:], acc[:, 0, :])
                else:
                    nc.vector.tensor_copy(ot[:], acc[:, 0, :])
                nc.sync.dma_start(out_view[:, j, :], ot[:])

    nc.compile()
    _BUILT["nc"] = nc
    return _BUILT


def _in_maps(input_a, input_b, Wa, ba, Wb, bb):
    """Per-core input bindings: core 2b -> output_a[b], core 2b+1 -> output_b[b]."""
    c = np.ascontiguousarray
    maps = []
    for b in range(B):
        maps.append({  # role output_a: U=input_b, V=input_a
            "U": c(input_b[b]), "V": c(input_a[b]),
            "Wu": c(Wb), "bu": c(bb), "Wv": c(Wa), "bv": c(ba),
        })
        maps.append({  # role output_b: U=input_a, V=input_b
            "U": c(input_a[b]), "V": c(input_b[b]),
            "Wu": c(Wa), "bu": c(ba), "Wv": c(Wb), "bv": c(bb),
        })
    return maps


def run_on_hw(input_a, input_b, Wa, ba, Wb, bb, **run_kwargs):
    built = _build()
    maps = _in_maps(input_a, input_b, Wa, ba, Wb, bb)
    res = run_bass_kernel_spmd(built["nc"], maps, core_ids=list(range(8)), **run_kwargs)
    return res


def kernel(input_a, input_b, Wa, ba, Wb, bb, padding_values):
    input_a = np.asarray(input_a, np.float32)
    input_b = np.asarray(input_b, np.float32)
    res = run_on_hw(
        input_a, input_b,
        np.asarray(Wa, np.float32), np.asarray(ba, np.float32),
        np.asarray(Wb, np.float32), np.asarray(bb, np.float32),
    )
    out = np.empty((B, L, 3 * D), np.float32)
    for b in range(B):
        out[b, :, 0:D] = res.results[2 * b]["out"]
        out[b, :, D:2 * D] = np.asarray(padding_values[b], np.float32)
        out[b, :, 2 * D:3 * D] = res.results[2 * b + 1]["out"]
    return out

